# revision 23
# baseline (speedup 1.0000x reference)
"""HolE scorer kernel for 8 Trainium2 NeuronCores (Bass/Tile).

Computation (reference):
    a = x @ W_e.T; b = y @ W_e.T; rr = r @ W_r.T          # (B, d)
    corr = irfft(rfft(a) * conj(rfft(b))) / d             # circular correlation
    out = sigmoid(sum(rr * corr, axis=1))                 # (B, 1)

Strategy (v2):
  - Tensor-parallel over entities: core c holds entity rows
    [c*12500, (c+1)*12500) of x.T, y.T, W_e.T (padded to 12544 = 98*128).
  - Six sub-passes (y then x; batch columns 0:512, 512:768, 768:1024),
    each a full k-sweep producing partial d-major outputs, followed by a
    per-sub-pass ReduceScatter(add).  Finer trailing sub-passes keep the
    last exposed RS small (~256KB) and let earlier RSes + tail rffts
    hide under later sub-pass compute.  Core c owns batch rows
    {c*64..+63} u {512+c*32..+31} u {768+c*32..+31}.
  - Tail per core (128 batch rows): rr.T GEMM + rffts via DFT-basis
    matmuls; irfft+rowwise-dot folded into a frequency-domain weighted
    dot (Parseval):  score = reduce_sum(R' . (A * conj(B))) with the
    w/d^2 factor folded into the R basis.
  - All matmul inputs bf16 (fp32 PSUM accumulation).
  - DMA: sync queue carries the k-streams with weight-group pieces
    interleaved 1:1 so stream tiles are never starved behind the
    resident-weight prefetch; first group is loaded j-granular so the
    first matmul starts ~6us in.  Scalar queue carries static tensors,
    stage-out copies and tail loads.
"""

import numpy as np
import ml_dtypes

import concourse.bass as bass
import concourse.tile as tile
from concourse import bacc, mybir
from concourse.alu_op_type import AluOpType
from concourse.bass_utils import run_bass_kernel_spmd

# Problem shapes (hardcoded per contract)
B = 1024            # batch
D = 512             # num_dim
E = 100000          # num_entities
R = 1000            # num_relations
NCORES = 8

E_SH = E // NCORES          # 12500 entities per core
KC = 98                     # k-chunks of 128 after padding (98*128 = 12544)
E_PAD = KC * 128            # 12544
KG = 7                      # weight k-groups (resident tiles)
KJ = KC // KG               # 14 chunks per weight group
SG = 14                     # stream groups (7 k-chunks each)
SJ = KC // SG               # 7
RC = 8                      # relation k-chunks (1000 -> 1024)
R_PAD = RC * 128
NF = D // 2 + 1             # 257 rfft bins
B_SH = B // NCORES          # 128 batch rows per core

# sub-pass column blocks: (col0, width).  Tail-chunk width per core = W/8.
BLOCKS = [(0, 512), (512, 256), (768, 256)]
CHUNKW = [w // NCORES for _, w in BLOCKS]          # 64, 32, 32
CHUNKO = [0, 64, 96]                               # tail partition offsets

BF16 = mybir.dt.bfloat16
F32 = mybir.dt.float32

_cached = {}


def _dft_bases():
    d = D
    dd = np.arange(d, dtype=np.float64)[:, None]
    ff = np.arange(NF, dtype=np.float64)[None, :]
    ang = 2.0 * np.pi * dd * ff / d
    fr = np.cos(ang)
    fi = -np.sin(ang)
    f_ab = np.concatenate([fr, fi], axis=1)              # (512, 514)
    w = np.full(NF, 2.0); w[0] = 1.0; w[-1] = 1.0
    scale = w / (d * d)
    f_r = np.concatenate([fr * scale, fi * scale], axis=1)
    return (f_ab.astype(ml_dtypes.bfloat16), f_r.astype(ml_dtypes.bfloat16))


def _build_program():
    nc = bacc.Bacc("TRN2", target_bir_lowering=False, debug=False,
                   num_devices=NCORES)

    xT_d = nc.dram_tensor("xT", (E_PAD, B), BF16, kind="ExternalInput")
    yT_d = nc.dram_tensor("yT", (E_PAD, B), BF16, kind="ExternalInput")
    weT_d = nc.dram_tensor("weT", (E_PAD, D), BF16, kind="ExternalInput")
    rT_d = nc.dram_tensor("rT", (R_PAD, B_SH), BF16, kind="ExternalInput")
    wrT_d = nc.dram_tensor("wrT", (R_PAD, D), BF16, kind="ExternalInput")
    fab_d = nc.dram_tensor("fab", (D, 2 * NF), BF16, kind="ExternalInput")
    fr_d = nc.dram_tensor("fr", (D, 2 * NF), BF16, kind="ExternalInput")
    out_d = nc.dram_tensor("out", (B_SH, 1), F32, kind="ExternalOutput")

    # per-(tensor, block) staging + reduce-scatter outputs
    stages = {}
    rs_outs = {}
    for mat in ("b", "a"):
        for blk, (c0, w) in enumerate(BLOCKS):
            cw = w // NCORES
            stages[(mat, blk)] = nc.dram_tensor(
                f"stage_{mat}{blk}", (NCORES, D, cw), BF16)
            rs_outs[(mat, blk)] = nc.dram_tensor(
                f"rs_{mat}{blk}", (D, cw), BF16)
    groups = [list(range(NCORES))]

    with tile.TileContext(nc) as tc:
        with (
            tc.tile_pool(name="weights", bufs=1) as wpool,
            tc.tile_pool(name="stream", bufs=3) as spool,
            tc.tile_pool(name="copies", bufs=4) as cpool,
            tc.tile_pool(name="tail", bufs=1) as tpool,
            tc.tile_pool(name="psum", bufs=8, space="PSUM") as ppool,
        ):
            # small static tensors; DMA'd on the sync queue AFTER the first
            # two stream groups (emitted in the main loop below) so they
            # don't compete with the startup-critical loads.
            r_t = wpool.tile([128, RC, B_SH], BF16, tag="r", name="r")
            wr_t = wpool.tile([128, RC, D], BF16, tag="wr", name="wr")
            fab_t = wpool.tile([128, 4, 2 * NF], BF16, tag="fab", name="fab")
            fr_t = wpool.tile([128, 4, 2 * NF], BF16, tag="frq", name="frq")

            def load_statics():
                nc.sync.dma_start(
                    r_t[:], rT_d[:].rearrange("(j p) q -> p j q", p=128))
                nc.sync.dma_start(
                    wr_t[:], wrT_d[:].rearrange("(j p) q -> p j q", p=128))
                nc.sync.dma_start(
                    fab_t[:], fab_d[:].rearrange("(j p) q -> p j q", p=128))
                nc.sync.dma_start(
                    fr_t[:], fr_d[:].rearrange("(j p) q -> p j q", p=128))

            # resident W_e.T tiles; DMA emission is interleaved with the
            # first sub-pass's stream loads below (same sync queue).
            we_tiles = [
                wpool.tile([128, KJ, D], BF16, tag=f"we{g}", name=f"we{g}")
                for g in range(KG)
            ]

            def we_src(g):
                return (weT_d[g * KJ * 128:(g + 1) * KJ * 128, :]
                        .rearrange("(j p) q -> p j q", p=128))

            def load_we_piece(G):
                """Stream-group G covers k-chunks [G*7, G*7+7) = weight
                tile G//2, j-half G%2.  Weights share the sync ring with
                the stream, interleaved 1:1 per group — the FIFO acts as a
                rate limiter so prefetch can't starve the stream."""
                g, h = G // 2, G % 2
                sl = slice(h * SJ, (h + 1) * SJ)
                nc.sync.dma_start(we_tiles[g][:, sl], we_src(g)[:, sl])

            rr_b = tpool.tile([128, 4, B_SH], BF16, name="rr_b")
            s_qr = tpool.tile([B_SH, NF], F32, name="s_qr")
            s_qi = tpool.tile([B_SH, NF], F32, name="s_qi")
            f1 = tpool.tile([B_SH, NF], F32, name="f1")
            f2 = tpool.tile([B_SH, NF], F32, name="f2")
            g_t = tpool.tile([B_SH, 2 * NF], F32, name="g_t")
            sig = tpool.tile([B_SH, 1], F32, name="sig")

            def rfft_mm(src_b, basis, psr, psi, lo, w):
                for k in range(4):
                    nc.tensor.matmul(psr[lo:lo + w, :], src_b[:, k, :],
                                     basis[:, k, 0:NF],
                                     start=(k == 0), stop=(k == 3),
                                     tile_position=(0, lo))
                for k in range(4):
                    nc.tensor.matmul(psi[lo:lo + w, :], src_b[:, k, :],
                                     basis[:, k, NF:2 * NF],
                                     start=(k == 0), stop=(k == 3),
                                     tile_position=(0, lo))

            def load_chunk(mat, blk):
                cw = CHUNKW[blk]
                tb = tpool.tile([128, 4, cw], BF16, name=f"{mat}T{blk}")
                nc.scalar.dma_start(
                    tb[:],
                    rs_outs[(mat, blk)][:]
                    .rearrange("(mc p) q -> p mc q", p=128))
                return tb

            ps_br = None
            ps_bi = None
            ps_ar = None
            ps_ai = None

            def a_chunk_tail(blk):
                """rfft the a-chunk for block blk, multiply by F factors,
                rowwise-reduce, sigmoid, and DMA out its rows."""
                lo, cw = CHUNKO[blk], CHUNKW[blk]
                a_half = load_chunk("a", blk)
                rfft_mm(a_half, fab_t, ps_ar, ps_ai, lo, cw)
                sl = slice(lo, lo + cw)
                nc.vector.tensor_tensor(g_t[sl, 0:NF], ps_ar[sl], f1[sl],
                                        AluOpType.mult)
                nc.vector.tensor_tensor(g_t[sl, NF:2 * NF], ps_ai[sl],
                                        f2[sl], AluOpType.mult)
                score = tpool.tile([cw, 1], F32, tag="score",
                                   name=f"score{blk}")
                nc.vector.reduce_sum(score[:], g_t[sl, :],
                                     axis=mybir.AxisListType.X)
                nc.scalar.activation(sig[sl], score[:],
                                     mybir.ActivationFunctionType.Sigmoid)
                nc.sync.dma_start(out_d[lo:lo + cw, :], sig[sl])

            # ---- main sub-passes ----
            sub_passes = [("b", yT_d, 0), ("b", yT_d, 1), ("b", yT_d, 2),
                          ("a", xT_d, 0), ("a", xT_d, 1), ("a", xT_d, 2)]
            for pi_, (mat, mat_d, blk) in enumerate(sub_passes):
                c0, W = BLOCKS[blk]
                accs = [
                    ppool.tile([128, W], F32, tag="acc",
                               name=f"acc{mat}{blk}{m}")
                    for m in range(4)
                ]
                for G in range(SG):
                    xt = spool.tile([128, SJ, W], BF16, tag=f"xs{W}",
                                    name=f"xs{mat}{blk}{G}",
                                    bufs=6)
                    src = (mat_d[G * SJ * 128:(G + 1) * SJ * 128,
                                 c0:c0 + W]
                           .rearrange("(j p) q -> p j q", p=128))
                    if pi_ == 0 and G == 0:
                        # j-interleave weight and stream pieces so the
                        # first matmul's inputs land first.
                        nc.sync.dma_start(we_tiles[0][:, 0:1, 0:128],
                                          we_src(0)[:, 0:1, 0:128])
                        nc.sync.dma_start(xt[:, 0:1], src[:, 0:1])
                        nc.sync.dma_start(we_tiles[0][:, 0:1, 128:D],
                                          we_src(0)[:, 0:1, 128:D])
                        for j in range(1, SJ):
                            nc.sync.dma_start(we_tiles[0][:, j:j + 1],
                                              we_src(0)[:, j:j + 1])
                            nc.sync.dma_start(xt[:, j:j + 1],
                                              src[:, j:j + 1])
                    else:
                        if pi_ == 0:
                            load_we_piece(G)
                        nc.sync.dma_start(xt[:], src)
                    if pi_ == 0 and G == 8:
                        load_statics()
                    if pi_ == 5 and G == 10:
                        a_chunk_tail(1)
                    for j in range(SJ):
                        k = G * SJ + j
                        gw, jj = k // KJ, k % KJ
                        for m in range(4):
                            nc.tensor.matmul(
                                accs[m][:],
                                we_tiles[gw][:, jj, m * 128:(m + 1) * 128],
                                xt[:, j, :],
                                start=(k == 0), stop=(k == KC - 1))
                # fold to bf16 + stage out + reduce-scatter.  For the final
                # sub-pass the sync queue is idle, so alternate queues to
                # parallelize descriptor generation on the critical path.
                cw = W // NCORES
                for m in range(4):
                    sb = cpool.tile([128, W], BF16, tag=f"cp{W}",
                                    name=f"cp{mat}{blk}{m}")
                    nc.vector.tensor_copy(sb[:], accs[m][:])
                    dst = (stages[(mat, blk)][:, m * 128:(m + 1) * 128, :]
                           .rearrange("c d q -> d c q"))
                    q = nc.sync if (pi_ == 5 and m % 2) else nc.scalar
                    q.dma_start(
                        dst, sb.rearrange("d (c q) -> d c q", c=NCORES))
                nc.gpsimd.collective_compute(
                    "ReduceScatter", AluOpType.add,
                    replica_groups=groups,
                    ins=[stages[(mat, blk)][:].opt()],
                    outs=[rs_outs[(mat, blk)][:].opt()])

                if pi_ == 0:
                    # rr.T GEMM + rr-rfft, hidden behind sub-pass 1+.
                    ps_rr = ppool.tile([128, 4, B_SH], F32, tag="acc",
                                       name="ps_rr")
                    for m in range(4):
                        for j in range(RC):
                            nc.tensor.matmul(
                                ps_rr[:, m, :],
                                wr_t[:, j, m * 128:(m + 1) * 128],
                                r_t[:, j, :],
                                start=(j == 0), stop=(j == RC - 1))
                    nc.vector.tensor_copy(rr_b[:], ps_rr[:])
                    ps_qr = ppool.tile([B_SH, NF], F32, tag="acc",
                                       name="ps_qr")
                    ps_qi = ppool.tile([B_SH, NF], F32, tag="acc",
                                       name="ps_qi")
                    rfft_mm(rr_b, fr_t, ps_qr, ps_qi, 0, B_SH)
                    nc.vector.tensor_copy(s_qr[:], ps_qr[:])
                    nc.vector.tensor_copy(s_qi[:], ps_qi[:])

                if pi_ == 3:
                    # all b rfft chunks + F factors (all y RSes done by
                    # the time sub-pass 3's k-sweep ends).
                    ps_br = ppool.tile([B_SH, NF], F32, tag="acc",
                                       name="ps_br")
                    ps_bi = ppool.tile([B_SH, NF], F32, tag="acc",
                                       name="ps_bi")
                    for bb in (0, 1, 2):
                        bt = load_chunk("b", bb)
                        rfft_mm(bt, fab_t, ps_br, ps_bi,
                                CHUNKO[bb], CHUNKW[bb])
                    t1 = tpool.tile([B_SH, NF], F32, name="t1")
                    t2 = tpool.tile([B_SH, NF], F32, name="t2")
                    nc.vector.tensor_tensor(f1[:], ps_br[:], s_qr[:],
                                            AluOpType.mult)
                    nc.vector.tensor_tensor(t1[:], ps_bi[:], s_qi[:],
                                            AluOpType.mult)
                    nc.vector.tensor_tensor(f1[:], f1[:], t1[:],
                                            AluOpType.subtract)
                    nc.vector.tensor_tensor(f2[:], ps_bi[:], s_qr[:],
                                            AluOpType.mult)
                    nc.vector.tensor_tensor(t2[:], ps_br[:], s_qi[:],
                                            AluOpType.mult)
                    nc.vector.tensor_tensor(f2[:], f2[:], t2[:],
                                            AluOpType.add)

                if pi_ == 4:
                    ps_ar = ppool.tile([B_SH, NF], F32, tag="acc",
                                       name="ps_ar")
                    ps_ai = ppool.tile([B_SH, NF], F32, tag="acc",
                                       name="ps_ai")
                    a_chunk_tail(0)

            # ---- exposed tail: a chunk 2 only ----
            a_chunk_tail(2)

    nc.compile()
    return nc


def _get_program():
    if "nc" not in _cached:
        _cached["nc"] = _build_program()
    return _cached["nc"]


def _core_rows(c):
    """Batch rows owned by core c (order matches tail layout)."""
    return np.r_[c * 64:(c + 1) * 64,
                 512 + c * 32:512 + (c + 1) * 32,
                 768 + c * 32:768 + (c + 1) * 32]


def kernel(x, y, r, W_e, W_r):
    nc = _get_program()
    bf = ml_dtypes.bfloat16

    f_ab, f_r = _dft_bases()

    wrT = np.zeros((R_PAD, D), dtype=bf)
    wrT[:R, :] = W_r.astype(bf).T
    rT_pad = np.zeros((R_PAD, B), dtype=bf)
    rT_pad[:R, :] = np.ascontiguousarray(r.T).astype(bf)

    xT = np.ascontiguousarray(x.T).astype(bf)     # (E, B)
    yT = np.ascontiguousarray(y.T).astype(bf)
    weT = np.ascontiguousarray(W_e.T).astype(bf)  # (E, D)

    in_maps = []
    for c in range(NCORES):
        lo, hi = c * E_SH, (c + 1) * E_SH
        xT_sh = np.zeros((E_PAD, B), dtype=bf)
        xT_sh[:E_SH] = xT[lo:hi]
        yT_sh = np.zeros((E_PAD, B), dtype=bf)
        yT_sh[:E_SH] = yT[lo:hi]
        weT_sh = np.zeros((E_PAD, D), dtype=bf)
        weT_sh[:E_SH] = weT[lo:hi]
        in_maps.append({
            "xT": xT_sh,
            "yT": yT_sh,
            "weT": weT_sh,
            "rT": np.ascontiguousarray(rT_pad[:, _core_rows(c)]),
            "wrT": wrT,
            "fab": f_ab,
            "fr": f_r,
        })

    res = run_bass_kernel_spmd(nc, in_maps, core_ids=list(range(NCORES)))
    out = np.empty((B, 1), dtype=np.float32)
    for c in range(NCORES):
        out[_core_rows(c)] = res.results[c]["out"]
    return out


# revision 24
# speedup vs baseline: 1.0207x; 1.0207x over previous
"""HolE scorer kernel for 8 Trainium2 NeuronCores (Bass/Tile).

Computation (reference):
    a = x @ W_e.T; b = y @ W_e.T; rr = r @ W_r.T          # (B, d)
    corr = irfft(rfft(a) * conj(rfft(b))) / d             # circular correlation
    out = sigmoid(sum(rr * corr, axis=1))                 # (B, 1)

Strategy (v2):
  - Tensor-parallel over entities: core c holds entity rows
    [c*12500, (c+1)*12500) of x.T, y.T, W_e.T (padded to 12544 = 98*128).
  - Six sub-passes (y then x; batch columns 0:512, 512:768, 768:1024),
    each a full k-sweep producing partial d-major outputs, followed by a
    per-sub-pass ReduceScatter(add).  Finer trailing sub-passes keep the
    last exposed RS small (~256KB) and let earlier RSes + tail rffts
    hide under later sub-pass compute.  Core c owns batch rows
    {c*64..+63} u {512+c*32..+31} u {768+c*32..+31}.
  - Tail per core (128 batch rows): rr.T GEMM + rffts via DFT-basis
    matmuls; irfft+rowwise-dot folded into a frequency-domain weighted
    dot (Parseval):  score = reduce_sum(R' . (A * conj(B))) with the
    w/d^2 factor folded into the R basis.
  - All matmul inputs bf16 (fp32 PSUM accumulation).
  - DMA: sync queue carries the k-streams with weight-group pieces
    interleaved 1:1 so stream tiles are never starved behind the
    resident-weight prefetch; first group is loaded j-granular so the
    first matmul starts ~6us in.  Scalar queue carries static tensors,
    stage-out copies and tail loads.
"""

import numpy as np
import ml_dtypes

import concourse.bass as bass
import concourse.tile as tile
from concourse import bacc, mybir
from concourse.alu_op_type import AluOpType
from concourse.bass_utils import run_bass_kernel_spmd

# Problem shapes (hardcoded per contract)
B = 1024            # batch
D = 512             # num_dim
E = 100000          # num_entities
R = 1000            # num_relations
NCORES = 8

E_SH = E // NCORES          # 12500 entities per core
KC = 98                     # k-chunks of 128 after padding (98*128 = 12544)
E_PAD = KC * 128            # 12544
KG = 7                      # weight k-groups (resident tiles)
KJ = KC // KG               # 14 chunks per weight group
SG = 14                     # stream groups (7 k-chunks each)
SJ = KC // SG               # 7
RC = 8                      # relation k-chunks (1000 -> 1024)
R_PAD = RC * 128
NF = D // 2 + 1             # 257 rfft bins
B_SH = B // NCORES          # 128 batch rows per core

# sub-pass column blocks: (col0, width).  Tail-chunk width per core = W/8.
BLOCKS = [(0, 512), (512, 256), (768, 256)]
CHUNKW = [w // NCORES for _, w in BLOCKS]          # 64, 32, 32
CHUNKO = [0, 64, 96]                               # tail partition offsets

BF16 = mybir.dt.bfloat16
F32 = mybir.dt.float32

_cached = {}


def _dft_bases():
    d = D
    dd = np.arange(d, dtype=np.float64)[:, None]
    ff = np.arange(NF, dtype=np.float64)[None, :]
    ang = 2.0 * np.pi * dd * ff / d
    fr = np.cos(ang)
    fi = -np.sin(ang)
    f_ab = np.concatenate([fr, fi], axis=1)              # (512, 514)
    w = np.full(NF, 2.0); w[0] = 1.0; w[-1] = 1.0
    scale = w / (d * d)
    f_r = np.concatenate([fr * scale, fi * scale], axis=1)
    return (f_ab.astype(ml_dtypes.bfloat16), f_r.astype(ml_dtypes.bfloat16))


def _build_program():
    nc = bacc.Bacc("TRN2", target_bir_lowering=False, debug=False,
                   num_devices=NCORES)

    xT_d = nc.dram_tensor("xT", (E_PAD, B), BF16, kind="ExternalInput")
    yT_d = nc.dram_tensor("yT", (E_PAD, B), BF16, kind="ExternalInput")
    weT_d = nc.dram_tensor("weT", (E_PAD, D), BF16, kind="ExternalInput")
    rT_d = nc.dram_tensor("rT", (R_PAD, B_SH), BF16, kind="ExternalInput")
    wrT_d = nc.dram_tensor("wrT", (R_PAD, D), BF16, kind="ExternalInput")
    fab_d = nc.dram_tensor("fab", (D, 2 * NF), BF16, kind="ExternalInput")
    fr_d = nc.dram_tensor("fr", (D, 2 * NF), BF16, kind="ExternalInput")
    out_d = nc.dram_tensor("out", (B_SH, 1), F32, kind="ExternalOutput")

    # per-(tensor, block) staging + reduce-scatter outputs
    stages = {}
    rs_outs = {}
    for mat in ("b", "a"):
        for blk, (c0, w) in enumerate(BLOCKS):
            cw = w // NCORES
            stages[(mat, blk)] = nc.dram_tensor(
                f"stage_{mat}{blk}", (NCORES, D, cw), BF16)
            rs_outs[(mat, blk)] = nc.dram_tensor(
                f"rs_{mat}{blk}", (D, cw), BF16)
    groups = [list(range(NCORES))]

    with tile.TileContext(nc) as tc:
        with (
            tc.tile_pool(name="weights", bufs=1) as wpool,
            tc.tile_pool(name="stream", bufs=3) as spool,
            tc.tile_pool(name="copies", bufs=4) as cpool,
            tc.tile_pool(name="tail", bufs=1) as tpool,
            tc.tile_pool(name="psum", bufs=8, space="PSUM") as ppool,
        ):
            # small static tensors; DMA'd on the sync queue AFTER the first
            # two stream groups (emitted in the main loop below) so they
            # don't compete with the startup-critical loads.
            r_t = wpool.tile([128, RC, B_SH], BF16, tag="r", name="r")
            wr_t = wpool.tile([128, RC, D], BF16, tag="wr", name="wr")
            fab_t = wpool.tile([128, 4, 2 * NF], BF16, tag="fab", name="fab")
            fr_t = wpool.tile([128, 4, 2 * NF], BF16, tag="frq", name="frq")

            def load_statics():
                nc.sync.dma_start(
                    r_t[:], rT_d[:].rearrange("(j p) q -> p j q", p=128))
                nc.sync.dma_start(
                    wr_t[:], wrT_d[:].rearrange("(j p) q -> p j q", p=128))
                nc.sync.dma_start(
                    fab_t[:], fab_d[:].rearrange("(j p) q -> p j q", p=128))
                nc.sync.dma_start(
                    fr_t[:], fr_d[:].rearrange("(j p) q -> p j q", p=128))

            # resident W_e.T tiles; DMA emission is interleaved with the
            # first sub-pass's stream loads below (same sync queue).
            we_tiles = [
                wpool.tile([128, KJ, D], BF16, tag=f"we{g}", name=f"we{g}")
                for g in range(KG)
            ]

            def we_src(g):
                return (weT_d[g * KJ * 128:(g + 1) * KJ * 128, :]
                        .rearrange("(j p) q -> p j q", p=128))

            def load_we_piece(G):
                """Stream-group G covers k-chunks [G*7, G*7+7) = weight
                tile G//2, j-half G%2.  Weights share the sync ring with
                the stream, interleaved 1:1 per group — the FIFO acts as a
                rate limiter so prefetch can't starve the stream."""
                g, h = G // 2, G % 2
                sl = slice(h * SJ, (h + 1) * SJ)
                nc.sync.dma_start(we_tiles[g][:, sl], we_src(g)[:, sl])

            rr_b = tpool.tile([128, 4, B_SH], BF16, name="rr_b")
            s_qr = tpool.tile([B_SH, NF], F32, name="s_qr")
            s_qi = tpool.tile([B_SH, NF], F32, name="s_qi")
            f1 = tpool.tile([B_SH, NF], F32, name="f1")
            f2 = tpool.tile([B_SH, NF], F32, name="f2")
            g_t = tpool.tile([B_SH, 2 * NF], F32, name="g_t")
            sig = tpool.tile([B_SH, 1], F32, name="sig")

            def rfft_mm(src_b, basis, psr, psi, lo, w):
                for k in range(4):
                    nc.tensor.matmul(psr[lo:lo + w, :], src_b[:, k, :],
                                     basis[:, k, 0:NF],
                                     start=(k == 0), stop=(k == 3),
                                     tile_position=(0, lo))
                for k in range(4):
                    nc.tensor.matmul(psi[lo:lo + w, :], src_b[:, k, :],
                                     basis[:, k, NF:2 * NF],
                                     start=(k == 0), stop=(k == 3),
                                     tile_position=(0, lo))

            def load_chunk(mat, blk):
                cw = CHUNKW[blk]
                tb = tpool.tile([128, 4, cw], BF16, name=f"{mat}T{blk}")
                nc.scalar.dma_start(
                    tb[:],
                    rs_outs[(mat, blk)][:]
                    .rearrange("(mc p) q -> p mc q", p=128))
                return tb

            ps_br = None
            ps_bi = None
            ps_ar = None
            ps_ai = None

            def a_chunk_tail(blk):
                """rfft the a-chunk for block blk, multiply by F factors,
                rowwise-reduce, sigmoid, and DMA out its rows."""
                lo, cw = CHUNKO[blk], CHUNKW[blk]
                a_half = load_chunk("a", blk)
                rfft_mm(a_half, fab_t, ps_ar, ps_ai, lo, cw)
                sl = slice(lo, lo + cw)
                nc.vector.tensor_tensor(g_t[sl, 0:NF], ps_ar[sl], f1[sl],
                                        AluOpType.mult)
                nc.vector.tensor_tensor(g_t[sl, NF:2 * NF], ps_ai[sl],
                                        f2[sl], AluOpType.mult)
                score = tpool.tile([cw, 1], F32, tag="score",
                                   name=f"score{blk}")
                nc.vector.reduce_sum(score[:], g_t[sl, :],
                                     axis=mybir.AxisListType.X)
                nc.scalar.activation(sig[sl], score[:],
                                     mybir.ActivationFunctionType.Sigmoid)
                nc.sync.dma_start(out_d[lo:lo + cw, :], sig[sl])

            # ---- main sub-passes ----
            sub_passes = [("b", yT_d, 0), ("b", yT_d, 1), ("b", yT_d, 2),
                          ("a", xT_d, 0), ("a", xT_d, 1), ("a", xT_d, 2)]
            for pi_, (mat, mat_d, blk) in enumerate(sub_passes):
                c0, W = BLOCKS[blk]
                accs = [
                    ppool.tile([128, W], F32, tag="acc",
                               name=f"acc{mat}{blk}{m}")
                    for m in range(4)
                ]
                for G in range(SG):
                    xt = spool.tile([128, SJ, W], BF16, tag=f"xs{W}",
                                    name=f"xs{mat}{blk}{G}",
                                    bufs=6)
                    src = (mat_d[G * SJ * 128:(G + 1) * SJ * 128,
                                 c0:c0 + W]
                           .rearrange("(j p) q -> p j q", p=128))
                    if pi_ == 0 and G == 0:
                        # j-interleave weight and stream pieces so the
                        # first matmul's inputs land first.
                        nc.sync.dma_start(we_tiles[0][:, 0:1, 0:128],
                                          we_src(0)[:, 0:1, 0:128])
                        nc.sync.dma_start(xt[:, 0:1], src[:, 0:1])
                        nc.sync.dma_start(we_tiles[0][:, 0:1, 128:D],
                                          we_src(0)[:, 0:1, 128:D])
                        for j in range(1, SJ):
                            nc.sync.dma_start(we_tiles[0][:, j:j + 1],
                                              we_src(0)[:, j:j + 1])
                            nc.sync.dma_start(xt[:, j:j + 1],
                                              src[:, j:j + 1])
                    else:
                        if pi_ == 0:
                            load_we_piece(G)
                        nc.sync.dma_start(xt[:], src)
                    if pi_ == 0 and G == 8:
                        load_statics()
                    for j in range(SJ):
                        k = G * SJ + j
                        gw, jj = k // KJ, k % KJ
                        for m in range(4):
                            nc.tensor.matmul(
                                accs[m][:],
                                we_tiles[gw][:, jj, m * 128:(m + 1) * 128],
                                xt[:, j, :],
                                start=(k == 0), stop=(k == KC - 1))
                # fold to bf16 + stage out + reduce-scatter.  For the final
                # sub-pass the sync queue is idle, so alternate queues to
                # parallelize descriptor generation on the critical path.
                cw = W // NCORES
                for m in range(4):
                    sb = cpool.tile([128, W], BF16, tag=f"cp{W}",
                                    name=f"cp{mat}{blk}{m}")
                    nc.vector.tensor_copy(sb[:], accs[m][:])
                    dst = (stages[(mat, blk)][:, m * 128:(m + 1) * 128, :]
                           .rearrange("c d q -> d c q"))
                    q = nc.sync if (pi_ == 5 and m % 2) else nc.scalar
                    q.dma_start(
                        dst, sb.rearrange("d (c q) -> d c q", c=NCORES))
                nc.gpsimd.collective_compute(
                    "ReduceScatter", AluOpType.add,
                    replica_groups=groups,
                    ins=[stages[(mat, blk)][:].opt()],
                    outs=[rs_outs[(mat, blk)][:].opt()])

                if pi_ == 0:
                    # rr.T GEMM + rr-rfft, hidden behind sub-pass 1+.
                    ps_rr = ppool.tile([128, 4, B_SH], F32, tag="acc",
                                       name="ps_rr")
                    for m in range(4):
                        for j in range(RC):
                            nc.tensor.matmul(
                                ps_rr[:, m, :],
                                wr_t[:, j, m * 128:(m + 1) * 128],
                                r_t[:, j, :],
                                start=(j == 0), stop=(j == RC - 1))
                    nc.vector.tensor_copy(rr_b[:], ps_rr[:])
                    ps_qr = ppool.tile([B_SH, NF], F32, tag="acc",
                                       name="ps_qr")
                    ps_qi = ppool.tile([B_SH, NF], F32, tag="acc",
                                       name="ps_qi")
                    rfft_mm(rr_b, fr_t, ps_qr, ps_qi, 0, B_SH)
                    nc.vector.tensor_copy(s_qr[:], ps_qr[:])
                    nc.vector.tensor_copy(s_qi[:], ps_qi[:])

                if pi_ == 3:
                    # all b rfft chunks + F factors (all y RSes done by
                    # the time sub-pass 3's k-sweep ends).
                    ps_br = ppool.tile([B_SH, NF], F32, tag="acc",
                                       name="ps_br")
                    ps_bi = ppool.tile([B_SH, NF], F32, tag="acc",
                                       name="ps_bi")
                    for bb in (0, 1, 2):
                        bt = load_chunk("b", bb)
                        rfft_mm(bt, fab_t, ps_br, ps_bi,
                                CHUNKO[bb], CHUNKW[bb])
                    t1 = tpool.tile([B_SH, NF], F32, name="t1")
                    t2 = tpool.tile([B_SH, NF], F32, name="t2")
                    nc.vector.tensor_tensor(f1[:], ps_br[:], s_qr[:],
                                            AluOpType.mult)
                    nc.vector.tensor_tensor(t1[:], ps_bi[:], s_qi[:],
                                            AluOpType.mult)
                    nc.vector.tensor_tensor(f1[:], f1[:], t1[:],
                                            AluOpType.subtract)
                    nc.vector.tensor_tensor(f2[:], ps_bi[:], s_qr[:],
                                            AluOpType.mult)
                    nc.vector.tensor_tensor(t2[:], ps_br[:], s_qi[:],
                                            AluOpType.mult)
                    nc.vector.tensor_tensor(f2[:], f2[:], t2[:],
                                            AluOpType.add)

                if pi_ == 4:
                    ps_ar = ppool.tile([B_SH, NF], F32, tag="acc",
                                       name="ps_ar")
                    ps_ai = ppool.tile([B_SH, NF], F32, tag="acc",
                                       name="ps_ai")
                    a_chunk_tail(0)

            # ---- exposed tail: a chunks 1, 2 ----
            a_chunk_tail(1)
            a_chunk_tail(2)

    nc.compile()
    return nc


def _get_program():
    if "nc" not in _cached:
        _cached["nc"] = _build_program()
    return _cached["nc"]


def _core_rows(c):
    """Batch rows owned by core c (order matches tail layout)."""
    return np.r_[c * 64:(c + 1) * 64,
                 512 + c * 32:512 + (c + 1) * 32,
                 768 + c * 32:768 + (c + 1) * 32]


def kernel(x, y, r, W_e, W_r):
    nc = _get_program()
    bf = ml_dtypes.bfloat16

    f_ab, f_r = _dft_bases()

    wrT = np.zeros((R_PAD, D), dtype=bf)
    wrT[:R, :] = W_r.astype(bf).T
    rT_pad = np.zeros((R_PAD, B), dtype=bf)
    rT_pad[:R, :] = np.ascontiguousarray(r.T).astype(bf)

    xT = np.ascontiguousarray(x.T).astype(bf)     # (E, B)
    yT = np.ascontiguousarray(y.T).astype(bf)
    weT = np.ascontiguousarray(W_e.T).astype(bf)  # (E, D)

    in_maps = []
    for c in range(NCORES):
        lo, hi = c * E_SH, (c + 1) * E_SH
        xT_sh = np.zeros((E_PAD, B), dtype=bf)
        xT_sh[:E_SH] = xT[lo:hi]
        yT_sh = np.zeros((E_PAD, B), dtype=bf)
        yT_sh[:E_SH] = yT[lo:hi]
        weT_sh = np.zeros((E_PAD, D), dtype=bf)
        weT_sh[:E_SH] = weT[lo:hi]
        in_maps.append({
            "xT": xT_sh,
            "yT": yT_sh,
            "weT": weT_sh,
            "rT": np.ascontiguousarray(rT_pad[:, _core_rows(c)]),
            "wrT": wrT,
            "fab": f_ab,
            "fr": f_r,
        })

    res = run_bass_kernel_spmd(nc, in_maps, core_ids=list(range(NCORES)))
    out = np.empty((B, 1), dtype=np.float32)
    for c in range(NCORES):
        out[_core_rows(c)] = res.results[c]["out"]
    return out
